# revision 10
# baseline (speedup 1.0000x reference)
"""Trainium2 Bass kernel for the pairwise-MLP adjacency module.

Computes out[b,i,j] = softmax_j( MLP(|v[b,i,:] - v[b,j,:]|) ) where the MLP is
128 -> 64 (leaky 0.1) -> 32 (leaky 0.1) -> 1, implemented as 1x1 convs in the
reference.

Sharding: 8 cores, 2 cores per batch element b (B=4); each core computes 256
of the 512 softmax rows for its b. Weights are replicated, packed host-side.

Per-core dataflow (v3, software-pipelined):
  - DVE: phi_i = relu(VT - v_i) as one fused tensor_scalar (sub + max0).
  - PE L1: z1 = 2*W1@relu(d) - W1@x + (W1@v_i + b1); two rows share a
    (128,512) PSUM tile (partition halves), the -W1@x term is one shared
    M=128 matmul (m1t), and the per-row constant rides the Prelu bias.
  - ACT: leaky-relu + bias fused into every PSUM->SBUF move (Prelu, 0.1).
  - PE L2: block-diag2(W2T) processes 2 rows per matmul; two consecutive
    steps write the two (128,512) halves of a (128,1024) 2-bank PSUM tile.
  - ACT: ONE Prelu evacuates each (128,1024) h2 pair (b2 bias is constant
    across rows, so pairing is legal; amortizes the ScalarE fixed cost).
  - PE L3: 8 shifted zero-padded copies of block-diag4(W3T) accumulate logits
    for 128 rows densely into one PSUM bank (4 col-groups x 8 shifts).
  - ACT: softmax via single Exp with fused row-sum (accum_out); DVE
    reciprocal + scale.  b3 dropped (softmax shift-invariant).
  - The PE stream at step e is [6x L1(e+2), 2x L2(e), L3(e-2)] so every
    matmul's inputs are ready well before issue (minimizes PE stalls, which
    otherwise hold the tensor engine in its low-clock pstate).
"""

import sys

for _p in ("/opt/trn_rl_repo",):
    if _p not in sys.path:
        sys.path.insert(0, _p)

from contextlib import ExitStack

import numpy as np
import ml_dtypes

import concourse.bass as bass
import concourse.bacc as bacc
import concourse.tile as tile
from concourse import mybir
from concourse.bass_utils import run_bass_kernel_spmd

BF16 = ml_dtypes.bfloat16

B, N, D = 4, 512, 128
H1, H2 = 64, 32
SLOPE = 0.1
NCORES = 8
IPC = B * N // NCORES      # 256 rows per core
NQ = IPC // 4              # 64 steps of 4 rows
QPB = NQ // 2              # 32 steps per softmax batch of 128 rows


def build_nc():
    f32 = mybir.dt.float32
    bf = mybir.dt.bfloat16
    nc = bacc.Bacc("TRN2", target_bir_lowering=False, debug=False)

    vt = nc.dram_tensor("vt", [D, N], bf, kind="ExternalInput").ap()
    vtq = nc.dram_tensor("vtq", [D, IPC], f32, kind="ExternalInput").ap()
    w1t2 = nc.dram_tensor("w1t2", [D, H1], bf, kind="ExternalInput").ap()
    m1t = nc.dram_tensor("m1t", [D, 128], bf, kind="ExternalInput").ap()
    w2bd = nc.dram_tensor("w2bd", [2 * H1, 2 * H2], bf, kind="ExternalInput").ap()
    w3v = nc.dram_tensor("w3v", [128, 8 * H2], bf, kind="ExternalInput").ap()
    cbias = nc.dram_tensor("cbias", [128, IPC // 2], f32, kind="ExternalInput").ap()
    b2s = nc.dram_tensor("b2s", [128, 1], f32, kind="ExternalInput").ap()
    outd = nc.dram_tensor("out", [IPC, N], f32, kind="ExternalOutput").ap()

    LR = mybir.ActivationFunctionType.Prelu  # parametric relu: reads alpha
    EXP = mybir.ActivationFunctionType.Exp
    SUB = mybir.AluOpType.subtract
    MAX = mybir.AluOpType.max

    with tile.TileContext(nc) as tc, ExitStack() as ctx:
        singles = ctx.enter_context(tc.tile_pool(name="singles", bufs=1))
        phip = ctx.enter_context(tc.tile_pool(name="phip", bufs=16))
        h1p = ctx.enter_context(tc.tile_pool(name="h1p", bufs=4))
        h2p = ctx.enter_context(tc.tile_pool(name="h2p", bufs=2))
        p1p = ctx.enter_context(tc.tile_pool(name="p1p", bufs=2, space="PSUM"))
        p2p = ctx.enter_context(tc.tile_pool(name="p2p", bufs=2, space="PSUM"))
        lgp = ctx.enter_context(tc.tile_pool(name="lgp", bufs=2, space="PSUM"))
        postp = ctx.enter_context(tc.tile_pool(name="postp", bufs=2))

        vt_sb = singles.tile([D, N], bf)
        nc.sync.dma_start(out=vt_sb, in_=vt)
        vtq_sb = singles.tile([D, IPC], f32)
        nc.sync.dma_start(out=vtq_sb, in_=vtq)
        w1_sb = singles.tile([D, H1], bf)
        nc.sync.dma_start(out=w1_sb, in_=w1t2)
        m1_sb = singles.tile([D, 128], bf)
        nc.sync.dma_start(out=m1_sb, in_=m1t)
        w2_sb = singles.tile([2 * H1, 2 * H2], bf)
        nc.sync.dma_start(out=w2_sb, in_=w2bd)
        w3_sb = singles.tile([128, 8 * H2], bf)
        nc.sync.dma_start(out=w3_sb, in_=w3v)
        cb_sb = singles.tile([128, IPC // 2], f32)
        nc.sync.dma_start(out=cb_sb, in_=cbias)
        b2_sb = singles.tile([128, 1], f32)
        nc.sync.dma_start(out=b2_sb, in_=b2s)

        phis = {}    # e -> list of 4 phi tiles
        p1s = {}     # e -> [2 x (128,512) psum tiles]
        h1s = {}     # e -> [2 x (128,512) sbuf tiles]
        p2s = {}     # even e -> (128,1024) psum pair tile (e, e+1)
        h2s = {}     # even e -> (128,1024) sbuf pair tile
        lgs = {}     # ib -> (128,512) psum tile

        def stage_phi(e):
            i0 = 4 * e
            tiles = []
            for k in range(4):
                ph = phip.tile([D, N], bf, tag="phip")
                nc.vector.tensor_scalar(
                    out=ph, in0=vt_sb,
                    scalar1=vtq_sb[:, i0 + k : i0 + k + 1], scalar2=0.0,
                    op0=SUB, op1=MAX,
                )
                tiles.append(ph)
            phis[e] = tiles

        def stage_l1(e):
            tiles = phis.pop(e)
            ps = []
            # m1-first ordering batches same-weight matmuls so the PE does
            # only two weight switches for the whole L1 step (m1 then w1).
            for half in range(2):
                p1 = p1p.tile([128, N], f32, tag="p1")
                # one M=128 matmul seeds BOTH halves with -W1@x
                nc.tensor.matmul(
                    p1, m1_sb, vt_sb,
                    start=True, stop=False, skip_group_check=True,
                )
                ps.append(p1)
            for k in range(2):
                # k-major order alternates PSUM banks between consecutive
                # matmuls (same-bank back-to-back writes serialize).
                for half in range(2):
                    # per-element has_written semantics make the
                    # seed + per-half accumulate legal.
                    nc.tensor.matmul(
                        ps[half][64 * k : 64 * k + 64, :],
                        w1_sb, tiles[2 * half + k],
                        start=False, stop=True, skip_group_check=True,
                    )
            p1s[e] = ps

        def stage_h1(e):
            ps = p1s.pop(e)
            hs = []
            for half in range(2):
                tp = 2 * e + half  # global pair index
                h1 = h1p.tile([128, N], bf, tag="h1")
                nc.scalar.activation(
                    out=h1, in_=ps[half], func=LR,
                    bias=cb_sb[:, tp : tp + 1], scale=1.0, alpha=SLOPE,
                )
                hs.append(h1)
            h1s[e] = hs

        def stage_l2(e):
            hs = h1s.pop(e)
            if e % 2 == 0:
                p2_new = p2p.tile([128, 2 * N], f32, tag="p2")
                p2s[e] = p2_new
            p2 = p2s[e - (e % 2)]
            coff = N * (e % 2)
            for half in range(2):
                nc.tensor.matmul(
                    p2[64 * half : 64 * half + 64, coff : coff + N],
                    w2_sb, hs[half], start=True, stop=True,
                )

        def stage_h2pair(e_even):
            p2 = p2s.pop(e_even)
            h2 = h2p.tile([128, 2 * N], bf, tag="h2")
            nc.scalar.activation(
                out=h2, in_=p2, func=LR, bias=b2_sb, scale=1.0, alpha=SLOPE
            )
            h2s[e_even] = h2

        def stage_l3(e):
            ib, q = divmod(e, QPB)
            if q == 0:
                lg_new = lgp.tile([128, N], f32, tag="lg")
                lgs[ib] = lg_new
            lg = lgs[ib]
            c0, dsh = divmod(q, 8)
            h2 = h2s[e - (e % 2)]
            coff = N * (e % 2)
            nc.tensor.matmul(
                lg[32 * c0 : 32 * c0 + 32, :],
                w3_sb[:, 32 * dsh : 32 * dsh + 32],
                h2[:, coff : coff + N],
                start=(dsh == 0),
                stop=(dsh == 7),
                tile_position=(0, 32 * c0),
            )
            if e % 2 == 1:
                del h2s[e - 1]

        def stage_softmax(ib):
            lg = lgs.pop(ib)
            expo = postp.tile([128, N], f32, tag="expo")
            sums = postp.tile([128, 1], f32, tag="sums")
            nc.scalar.activation(out=expo, in_=lg, func=EXP, accum_out=sums)
            rs = postp.tile([128, 1], f32, tag="rs")
            nc.vector.reciprocal(rs, sums)
            res = postp.tile([128, N], f32, tag="res")
            nc.vector.tensor_scalar_mul(out=res, in0=expo, scalar1=rs)
            nc.sync.dma_start(out=outd[ib * 128 : (ib + 1) * 128, :], in_=res)

        # software-pipelined schedule
        for e in range(-2, NQ + 2):
            if 0 <= e + 2 < NQ:
                stage_phi(e + 2)
                stage_l1(e + 2)
            if 0 <= e + 1 < NQ:
                stage_h1(e + 1)
            if 0 <= e < NQ:
                stage_l2(e)
                if e % 2 == 1:
                    stage_h2pair(e - 1)
            if 0 <= e - 2 < NQ:
                stage_l3(e - 2)
                if (e - 2) % QPB == QPB - 1:
                    stage_softmax((e - 2) // QPB)
    return nc


def make_in_maps(v, W1, b1, W2, b2, W3):
    w1t2 = np.ascontiguousarray(2.0 * W1.T).astype(BF16)             # (128, 64)
    m1t = np.ascontiguousarray(
        np.concatenate([-W1.T, -W1.T], axis=1)
    ).astype(BF16)                                                   # (128, 128)
    w1bf = w1t2.astype(np.float32).T / 2.0                           # device W1
    w2bd = np.zeros((2 * H1, 2 * H2), np.float32)
    w2bd[0:H1, 0:H2] = W2.T
    w2bd[H1 : 2 * H1, H2 : 2 * H2] = W2.T
    w2bd = w2bd.astype(BF16)
    w3v = np.zeros((128, 8 * H2), np.float32)
    for dsh in range(8):
        for k in range(4):
            w3v[32 * k : 32 * k + 32, 32 * dsh + 4 * dsh + k] = W3[0, :]
    w3v = w3v.astype(BF16)
    b2st = np.tile(b2, 4).reshape(128, 1).astype(np.float32)

    vts = v.transpose(0, 2, 1)  # (B, D, N)
    in_maps = []
    for c in range(NCORES):
        b, io = c // 2, IPC * (c % 2)
        vt_c = np.ascontiguousarray(vts[b]).astype(BF16)
        vtq_c = np.ascontiguousarray(vt_c[:, io : io + IPC].astype(np.float32))
        # per-row L1 constant: W1@v_i + b1, stacked per pair into one column
        c2 = w1bf @ vtq_c + b1[:, None]                              # (64, IPC)
        cbias_c = np.empty((128, IPC // 2), np.float32)
        cbias_c[0:H1] = c2[:, 0::2]
        cbias_c[H1:128] = c2[:, 1::2]
        in_maps.append(
            {
                "vt": vt_c,
                "vtq": vtq_c,
                "w1t2": w1t2,
                "m1t": m1t,
                "w2bd": w2bd,
                "w3v": w3v,
                "cbias": cbias_c,
                "b2s": b2st,
            }
        )
    return in_maps


_NC_CACHE = [None]


def get_nc():
    if _NC_CACHE[0] is None:
        nc = build_nc()
        nc.finalize()  # runs the Bacc lowering passes (reg alloc, sem split)
        _NC_CACHE[0] = nc
    return _NC_CACHE[0]


def run(inputs, trace=False, **kw):
    nc = get_nc()
    in_maps = make_in_maps(
        inputs["v"], inputs["W1"], inputs["b1"], inputs["W2"], inputs["b2"],
        inputs["W3"],
    )
    res = run_bass_kernel_spmd(nc, in_maps, list(range(NCORES)), trace=trace, **kw)
    out = np.empty((B, N, N), np.float32)
    for c in range(NCORES):
        b, io = c // 2, IPC * (c % 2)
        out[b, io : io + IPC, :] = np.asarray(res.results[c]["out"], np.float32)
    return out, res


def kernel(**inputs):
    out, _ = run(inputs, trace=False)
    return out


# revision 14
# speedup vs baseline: 1.0420x; 1.0420x over previous
"""Trainium2 Bass kernel for the pairwise-MLP adjacency module.

Computes out[b,i,j] = softmax_j( MLP(|v[b,i,:] - v[b,j,:]|) ) where the MLP is
128 -> 64 (leaky 0.1) -> 32 (leaky 0.1) -> 1, implemented as 1x1 convs in the
reference.

Sharding: 8 cores, 2 cores per batch element b (B=4); each core computes 256
of the 512 softmax rows for its b. Weights are replicated, packed host-side.

Per-core dataflow (v3, software-pipelined):
  - DVE: phi_i = relu(VT - v_i) as one fused tensor_scalar (sub + max0).
  - PE L1: z1 = 2*W1@relu(d) - W1@x + (W1@v_i + b1); two rows share a
    (128,512) PSUM tile (partition halves), the -W1@x term is one shared
    M=128 matmul (m1t), and the per-row constant rides the Prelu bias.
  - ACT: leaky-relu + bias fused into every PSUM->SBUF move (Prelu, 0.1).
  - PE L2: block-diag2(W2T) processes 2 rows per matmul; two consecutive
    steps write the two (128,512) halves of a (128,1024) 2-bank PSUM tile.
  - ACT: ONE Prelu evacuates each (128,1024) h2 pair (b2 bias is constant
    across rows, so pairing is legal; amortizes the ScalarE fixed cost).
  - PE L3: 8 shifted zero-padded copies of block-diag4(W3T) accumulate logits
    for 128 rows densely into one PSUM bank (4 col-groups x 8 shifts).
  - ACT: softmax via single Exp with fused row-sum (accum_out); DVE
    reciprocal + scale.  b3 dropped (softmax shift-invariant).
  - The PE stream at step e is [6x L1(e+2), 2x L2(e), L3(e-2)] so every
    matmul's inputs are ready well before issue (minimizes PE stalls, which
    otherwise hold the tensor engine in its low-clock pstate).
"""

import sys

for _p in ("/opt/trn_rl_repo",):
    if _p not in sys.path:
        sys.path.insert(0, _p)

from contextlib import ExitStack

import numpy as np
import ml_dtypes

import concourse.bass as bass
import concourse.bacc as bacc
import concourse.tile as tile
from concourse import mybir
from concourse.bass_utils import run_bass_kernel_spmd

BF16 = ml_dtypes.bfloat16


def _register_prelu_bias_op():
    """Register a custom DVE op computing Prelu(in0 + s0) with slope imm2,
    used to offload part of the PSUM->SBUF h1 evacuation from the (busy)
    scalar engine to the vector engine.  Uses the documented custom-DVE
    extension point (concourse.dve_ops.OPS); the uops_sha pin is computed
    from the same lower() call DveOp.compile uses."""
    import numpy as np
    from concourse import dve_ops as dvo
    from concourse.dve_spec import Spec, Src0, C0, C2, maxx
    from concourse.dve_spec import lower as dve_lower
    from concourse.dve_uop import DveOpSpec

    name = "PRELU_BIAS_ANT"
    for op in dvo.OPS:
        if op.name == name:
            return op

    def _ref(in0, in1, c0, c1, c2):
        u = np.asarray(in0, np.float32) + c0
        return np.maximum(u, u * c2)

    u = Src0 + C0
    spec = Spec(body=maxx(u, u * C2), reference=_ref)
    op = dvo.DveOp(name, spec, subdim=False, uops_sha={})
    dvo.OPS.append(op)
    dvo.CUSTOM_DVE_SPECS[name] = spec
    dvo._SUB_OPCODE_FOR_NAME[name] = dvo._CUSTOM_DVE_ROW_BASE + len(dvo.OPS) - 1
    for ver in ("v3", "v4"):
        r = DveOpSpec(
            name=name,
            opcode=dvo.get_dve_sub_opcode(name),
            uops=dve_lower(spec, ver=ver),
            rd1_en=dvo.has_src1(spec),
        )
        op.uops_sha[ver] = r.sha(ver)
    return op


PRELU_BIAS = _register_prelu_bias_op()

B, N, D = 4, 512, 128
H1, H2 = 64, 32
SLOPE = 0.1
NCORES = 8
IPC = B * N // NCORES      # 256 rows per core
NQ = IPC // 4              # 64 steps of 4 rows
QPB = NQ // 2              # 32 steps per softmax batch of 128 rows


def build_nc():
    f32 = mybir.dt.float32
    bf = mybir.dt.bfloat16
    nc = bacc.Bacc("TRN2", target_bir_lowering=False, debug=False)

    vt = nc.dram_tensor("vt", [D, N], bf, kind="ExternalInput").ap()
    vtq = nc.dram_tensor("vtq", [D, IPC], f32, kind="ExternalInput").ap()
    w1t2 = nc.dram_tensor("w1t2", [D, H1], bf, kind="ExternalInput").ap()
    m1t = nc.dram_tensor("m1t", [D, 128], bf, kind="ExternalInput").ap()
    w2bd = nc.dram_tensor("w2bd", [2 * H1, 2 * H2], bf, kind="ExternalInput").ap()
    w3v = nc.dram_tensor("w3v", [128, 8 * H2], bf, kind="ExternalInput").ap()
    cbias = nc.dram_tensor("cbias", [128, IPC // 2], f32, kind="ExternalInput").ap()
    b2s = nc.dram_tensor("b2s", [128, 1], f32, kind="ExternalInput").ap()
    outd = nc.dram_tensor("out", [IPC, N], f32, kind="ExternalOutput").ap()

    LR = mybir.ActivationFunctionType.Prelu  # parametric relu: reads alpha
    EXP = mybir.ActivationFunctionType.Exp
    SUB = mybir.AluOpType.subtract
    MAX = mybir.AluOpType.max

    with tile.TileContext(nc) as tc, ExitStack() as ctx:
        singles = ctx.enter_context(tc.tile_pool(name="singles", bufs=1))
        phip = ctx.enter_context(tc.tile_pool(name="phip", bufs=16))
        h1p = ctx.enter_context(tc.tile_pool(name="h1p", bufs=6))
        h2p = ctx.enter_context(tc.tile_pool(name="h2p", bufs=2))
        p1p = ctx.enter_context(tc.tile_pool(name="p1p", bufs=4, space="PSUM"))
        p2p = ctx.enter_context(tc.tile_pool(name="p2p", bufs=1, space="PSUM"))
        lgp = ctx.enter_context(tc.tile_pool(name="lgp", bufs=2, space="PSUM"))
        postp = ctx.enter_context(tc.tile_pool(name="postp", bufs=2))

        vt_sb = singles.tile([D, N], bf)
        nc.sync.dma_start(out=vt_sb, in_=vt)
        vtq_sb = singles.tile([D, IPC], f32)
        nc.sync.dma_start(out=vtq_sb, in_=vtq)
        w1_sb = singles.tile([D, H1], bf)
        nc.sync.dma_start(out=w1_sb, in_=w1t2)
        m1_sb = singles.tile([D, 128], bf)
        nc.sync.dma_start(out=m1_sb, in_=m1t)
        w2_sb = singles.tile([2 * H1, 2 * H2], bf)
        nc.sync.dma_start(out=w2_sb, in_=w2bd)
        w3_sb = singles.tile([128, 8 * H2], bf)
        nc.sync.dma_start(out=w3_sb, in_=w3v)
        cb_sb = singles.tile([128, IPC // 2], f32)
        nc.sync.dma_start(out=cb_sb, in_=cbias)
        b2_sb = singles.tile([128, 1], f32)
        nc.sync.dma_start(out=b2_sb, in_=b2s)

        phis = {}    # e -> list of 4 phi tiles
        p1s = {}     # e -> [2 x (128,512) psum tiles]
        h1s = {}     # e -> [2 x (128,512) sbuf tiles]
        p2s = {}     # even e -> (128,1024) psum pair tile (e, e+1)
        h2s = {}     # even e -> (128,1024) sbuf pair tile
        lgs = {}     # ib -> (128,512) psum tile

        def stage_phi(e):
            i0 = 4 * e
            tiles = []
            for k in range(4):
                ph = phip.tile([D, N], bf, tag="phip")
                nc.vector.tensor_scalar(
                    out=ph, in0=vt_sb,
                    scalar1=vtq_sb[:, i0 + k : i0 + k + 1], scalar2=0.0,
                    op0=SUB, op1=MAX,
                )
                tiles.append(ph)
            phis[e] = tiles

        def stage_l1_batch(e):
            # L1 for steps e and e+1 (8 rows) in one same-weight-batched
            # burst: 4x m1 seeds, then 8x w1 accumulates.  Consecutive
            # matmuls alternate PSUM banks (same-bank back-to-back writes
            # serialize) and the PE does only two weight switches.
            tiles = phis.pop(e) + phis.pop(e + 1)
            ps = []
            for t in range(4):
                p1 = p1p.tile([128, N], f32, tag="p1")
                # one M=128 matmul seeds BOTH partition halves with -W1@x
                nc.tensor.matmul(
                    p1, m1_sb, vt_sb,
                    start=True, stop=False, skip_group_check=True,
                )
                ps.append(p1)
            for k in range(2):
                for t in range(4):
                    # per-element has_written semantics make the
                    # seed + per-half accumulate legal.
                    nc.tensor.matmul(
                        ps[t][64 * k : 64 * k + 64, :],
                        w1_sb, tiles[2 * t + k],
                        start=False, stop=True, skip_group_check=True,
                    )
            p1s[e] = ps[0:2]
            p1s[e + 1] = ps[2:4]

        def stage_h1(e):
            ps = p1s.pop(e)
            hs = []
            for half in range(2):
                tp = 2 * e + half  # global pair index
                h1 = h1p.tile([128, N], bf, tag="h1")
                if e % 2 == 1 and half == 1:
                    # offload one of four evacuations to the vector engine
                    nc.vector._custom_dve(
                        PRELU_BIAS, out=h1, in0=ps[half],
                        s0=cb_sb[:, tp : tp + 1], imm2=SLOPE,
                    )
                else:
                    nc.scalar.activation(
                        out=h1, in_=ps[half], func=LR,
                        bias=cb_sb[:, tp : tp + 1], scale=1.0, alpha=SLOPE,
                    )
                hs.append(h1)
            h1s[e] = hs

        def stage_l2_pair(e):
            # L2 for steps e (cols 0:512) and e+1 (cols 512:1024) of one
            # (128,1024) pair tile, interleaved so consecutive matmuls hit
            # different PSUM banks.
            hs0 = h1s.pop(e)
            hs1 = h1s.pop(e + 1)
            p2_new = p2p.tile([128, 2 * N], f32, tag="p2")
            p2s[e] = p2_new
            for half in range(2):
                nc.tensor.matmul(
                    p2_new[64 * half : 64 * half + 64, 0:N],
                    w2_sb, hs0[half], start=True, stop=True,
                )
                nc.tensor.matmul(
                    p2_new[64 * half : 64 * half + 64, N : 2 * N],
                    w2_sb, hs1[half], start=True, stop=True,
                )

        def stage_h2pair(e_even):
            p2 = p2s.pop(e_even)
            h2 = h2p.tile([128, 2 * N], bf, tag="h2")
            nc.scalar.activation(
                out=h2, in_=p2, func=LR, bias=b2_sb, scale=1.0, alpha=SLOPE
            )
            h2s[e_even] = h2

        def stage_l3(e):
            ib, q = divmod(e, QPB)
            if q == 0:
                lg_new = lgp.tile([128, N], f32, tag="lg")
                lgs[ib] = lg_new
            lg = lgs[ib]
            c0, dsh = divmod(q, 8)
            h2 = h2s[e - (e % 2)]
            coff = N * (e % 2)
            nc.tensor.matmul(
                lg[32 * c0 : 32 * c0 + 32, :],
                w3_sb[:, 32 * dsh : 32 * dsh + 32],
                h2[:, coff : coff + N],
                start=(dsh == 0),
                stop=(dsh == 7),
                tile_position=(0, 32 * c0),
            )
            if e % 2 == 1:
                del h2s[e - 1]

        def stage_softmax(ib):
            lg = lgs.pop(ib)
            expo = postp.tile([128, N], f32, tag="expo")
            sums = postp.tile([128, 1], f32, tag="sums")
            nc.scalar.activation(out=expo, in_=lg, func=EXP, accum_out=sums)
            rs = postp.tile([128, 1], f32, tag="rs")
            nc.vector.reciprocal(rs, sums)
            res = postp.tile([128, N], f32, tag="res")
            nc.vector.tensor_scalar_mul(out=res, in0=expo, scalar1=rs)
            nc.sync.dma_start(out=outd[ib * 128 : (ib + 1) * 128, :], in_=res)

        # software-pipelined schedule over step pairs
        for ee in range(-4, NQ + 3, 2):
            if 0 <= ee + 4 < NQ:
                stage_phi(ee + 4)
                stage_phi(ee + 5)
            if 0 <= ee + 2 < NQ:
                stage_l1_batch(ee + 2)
            if 0 <= ee < NQ:
                stage_h1(ee)
                stage_h1(ee + 1)
            if 0 <= ee - 2 < NQ:
                stage_l2_pair(ee - 2)
                stage_h2pair(ee - 2)
            if 0 <= ee - 4 < NQ:
                stage_l3(ee - 4)
                stage_l3(ee - 3)
                for e3 in (ee - 4, ee - 3):
                    if e3 % QPB == QPB - 1:
                        stage_softmax(e3 // QPB)
    return nc


def make_in_maps(v, W1, b1, W2, b2, W3):
    w1t2 = np.ascontiguousarray(2.0 * W1.T).astype(BF16)             # (128, 64)
    m1t = np.ascontiguousarray(
        np.concatenate([-W1.T, -W1.T], axis=1)
    ).astype(BF16)                                                   # (128, 128)
    w1bf = w1t2.astype(np.float32).T / 2.0                           # device W1
    w2bd = np.zeros((2 * H1, 2 * H2), np.float32)
    w2bd[0:H1, 0:H2] = W2.T
    w2bd[H1 : 2 * H1, H2 : 2 * H2] = W2.T
    w2bd = w2bd.astype(BF16)
    w3v = np.zeros((128, 8 * H2), np.float32)
    for dsh in range(8):
        for k in range(4):
            w3v[32 * k : 32 * k + 32, 32 * dsh + 4 * dsh + k] = W3[0, :]
    w3v = w3v.astype(BF16)
    b2st = np.tile(b2, 4).reshape(128, 1).astype(np.float32)

    vts = v.transpose(0, 2, 1)  # (B, D, N)
    in_maps = []
    for c in range(NCORES):
        b, io = c // 2, IPC * (c % 2)
        vt_c = np.ascontiguousarray(vts[b]).astype(BF16)
        vtq_c = np.ascontiguousarray(vt_c[:, io : io + IPC].astype(np.float32))
        # per-row L1 constant: W1@v_i + b1, stacked per pair into one column
        c2 = w1bf @ vtq_c + b1[:, None]                              # (64, IPC)
        cbias_c = np.empty((128, IPC // 2), np.float32)
        cbias_c[0:H1] = c2[:, 0::2]
        cbias_c[H1:128] = c2[:, 1::2]
        in_maps.append(
            {
                "vt": vt_c,
                "vtq": vtq_c,
                "w1t2": w1t2,
                "m1t": m1t,
                "w2bd": w2bd,
                "w3v": w3v,
                "cbias": cbias_c,
                "b2s": b2st,
            }
        )
    return in_maps


_NC_CACHE = [None]


def get_nc():
    if _NC_CACHE[0] is None:
        nc = build_nc()
        nc.finalize()  # runs the Bacc lowering passes (reg alloc, sem split)
        _NC_CACHE[0] = nc
    return _NC_CACHE[0]


def run(inputs, trace=False, **kw):
    nc = get_nc()
    in_maps = make_in_maps(
        inputs["v"], inputs["W1"], inputs["b1"], inputs["W2"], inputs["b2"],
        inputs["W3"],
    )
    res = run_bass_kernel_spmd(nc, in_maps, list(range(NCORES)), trace=trace, **kw)
    out = np.empty((B, N, N), np.float32)
    for c in range(NCORES):
        b, io = c // 2, IPC * (c % 2)
        out[b, io : io + IPC, :] = np.asarray(res.results[c]["out"], np.float32)
    return out, res


def kernel(**inputs):
    out, _ = run(inputs, trace=False)
    return out
